# revision 19
# baseline (speedup 1.0000x reference)
"""Autoformer forward pass as a Bass/Tile kernel for Trainium2, 8 NeuronCores,
pure data parallel over batch (B=128 -> 16 samples/core).

Host side precomputes constant matrices (DFT / moving-average / centering /
shift permutations), folds layernorm gammas into downstream weights, and
builds the tiny embedding unfolds. The device kernel does everything else in
fp32: projections, FFT autocorrelation via DFT matmuls, batched top-5 delay
selection (vector.max/max_index), delay aggregation via irfft(Vf*conj(Uf))
matmuls, series decomposition via (I-A) matmuls, FFN with exact erf Gelu.

Layouts per sample:
  tc (192 x C): tiles (128,C)+(64,C), time on partitions
  ct (512 x T): 4 tiles (128,T), channels on partitions
  fd (97 x C):  one tile, rfft bins on partitions
matmul: out[m,n] = sum_k lhsT[k,m] rhs[k,n], k on partitions.
"""
import sys, os
sys.path.insert(0, "/opt/trn_rl_repo")
os.environ.setdefault("MYCRO_LOCAL_CACHE", "1")

import numpy as np
from contextlib import ExitStack

import concourse.bass as bass
import concourse.bacc as bacc
import concourse.tile as tile
from concourse import mybir
from concourse.bass_utils import run_bass_kernel_spmd

F32 = mybir.dt.float32
AF = mybir.ActivationFunctionType
OP = mybir.AluOpType
AX = mybir.AxisListType

L, NF, DM, DFF, CO, NM = 192, 97, 512, 2048, 6, 5
PRED, LBL, MA, TOPK = 96, 96, 25, 5
NCORES, BPC = 8, 16
TCH = [(0, 128), (128, 64)]
CCH = [(0, 128), (128, 128), (256, 128), (384, 128)]


def _host_consts():
    t = np.arange(L)[:, None].astype(np.float64)
    f = np.arange(NF)[None, :].astype(np.float64)
    ang = 2.0 * np.pi * t * f / L
    FCOS = np.cos(ang).astype(np.float32)
    FSIN = (-np.sin(ang)).astype(np.float32)
    wgt = np.full((NF, 1), 2.0); wgt[0, 0] = 1.0; wgt[NF - 1, 0] = 1.0
    ang2 = 2.0 * np.pi * np.arange(NF)[:, None] * np.arange(L)[None, :] / L
    ICOS = (wgt / L * np.cos(ang2)).astype(np.float32)
    ISIN = (-wgt / L * np.sin(ang2)).astype(np.float32)
    A = np.zeros((L, L), dtype=np.float64)
    pad = (MA - 1) // 2
    for tt in range(L):
        for i in range(MA):
            s = min(max(tt + i - pad, 0), L - 1)
            A[tt, s] += 1.0 / MA
    A = A.astype(np.float32)
    ADT = (np.eye(L, dtype=np.float32) - A).T.copy()
    CEN = (np.eye(L, dtype=np.float32) - np.float32(1.0 / L)).astype(np.float32)
    R = []
    for j in range(3):
        Rj = np.zeros((L, L), dtype=np.float32)
        for tt in range(L):
            Rj[(tt + j - 1) % L, tt] = 1.0
        R.append(Rj)
    return dict(FCOS=FCOS, FSIN=FSIN, ICOS=ICOS, ISIN=ISIN, A=A, ADT=ADT,
                CEN=CEN, R0=R[0], R2=R[2], ID192=np.eye(L, dtype=np.float32),
                ID16=np.eye(16, dtype=np.float32),
                IOTA=np.tile(np.arange(L, dtype=np.float32), (BPC, 1)))


def _prep_host(x_enc, x_mark_enc, x_dec, x_mark_dec, params, hc):
    B = np.asarray(x_enc).shape[0]
    A = hc["A"]
    xe = np.asarray(x_enc, np.float32)
    mean_xe = xe.mean(axis=1, keepdims=True, dtype=np.float32)
    trend = np.einsum("ts,bsc->btc", A, xe).astype(np.float32)
    seasonal = (xe - trend).astype(np.float32)
    trend_init = np.concatenate(
        [trend[:, L - LBL:], np.broadcast_to(mean_xe, (B, PRED, CO))], axis=1)
    dec_in = np.concatenate(
        [seasonal[:, L - LBL:], np.zeros((B, PRED, CO), np.float32)], axis=1)

    def unfold(x, mark):
        xT = np.transpose(x, (0, 2, 1))
        rows = [np.roll(xT, 1 - j, axis=2) for j in range(3)]
        return np.ascontiguousarray(
            np.concatenate(rows + [np.transpose(mark, (0, 2, 1))], axis=1),
            np.float32)

    UENC = unfold(xe, np.asarray(x_mark_enc, np.float32))
    UDEC = unfold(dec_in, np.asarray(x_mark_dec, np.float32))
    TRI = np.ascontiguousarray(np.transpose(trend_init, (0, 2, 1)), np.float32)

    p = params
    g_enc = np.asarray(p["enc_norm"]["g"], np.float32)
    g_dec = np.asarray(p["dec_norm"]["g"], np.float32)

    com = {k: hc[k] for k in ("FCOS", "FSIN", "ICOS", "ISIN", "ADT", "CEN",
                              "R0", "R2", "ID192", "ID16", "IOTA")}

    def emb(pp):
        Wc = np.asarray(pp["Wc"], np.float32).reshape(18, DM)
        Wt = np.asarray(pp["Wt"], np.float32)
        return np.ascontiguousarray(np.concatenate([Wc, Wt], axis=0), np.float32)

    com["EMB_E"] = emb(p["enc_emb"])
    com["EMB_D"] = emb(p["dec_emb"])

    def attnw(ap, pre=None):
        d = {}
        for nm in ("Wq", "Wk", "Wv", "Wo"):
            w = np.asarray(ap[nm], np.float32)
            if pre is not None and nm in ("Wk", "Wv"):
                w = (pre[:, None] * w).astype(np.float32)
            d[nm.lower()] = np.ascontiguousarray(w)
        for nm in ("bq", "bk", "bv", "bo"):
            b = np.asarray(ap[nm], np.float32)
            assert not np.any(b), f"nonzero bias {nm} unsupported"
        return d

    for l in range(2):
        el = p["enc_layers"][l]
        for k, v in attnw(el["attn"]).items():
            com[f"e{l}_{k}"] = v
        com[f"e{l}_w1"] = np.ascontiguousarray(np.asarray(el["W1"], np.float32))
        com[f"e{l}_w2"] = np.ascontiguousarray(np.asarray(el["W2"], np.float32))
    dl = p["dec_layers"][0]
    for k, v in attnw(dl["self"]).items():
        com[f"ds_{k}"] = v
    for k, v in attnw(dl["cross"], pre=g_enc).items():
        com[f"dc_{k}"] = v
    com["d_w1"] = np.ascontiguousarray(np.asarray(dl["W1"], np.float32))
    com["d_w2"] = np.ascontiguousarray(np.asarray(dl["W2"], np.float32))
    com["d_wt"] = np.ascontiguousarray(np.asarray(dl["Wt"], np.float32))
    com["WPP"] = np.ascontiguousarray(
        (g_dec[:, None] * np.asarray(p["Wp"], np.float32)).astype(np.float32))
    assert not np.any(np.asarray(p["bp"], np.float32)), "nonzero bp unsupported"
    # enc_norm/dec_norm biases cancel mathematically under time-centering

    per_core = []
    for c in range(NCORES):
        s = slice(c * BPC, (c + 1) * BPC)
        per_core.append(dict(UENC=np.ascontiguousarray(UENC[s]),
                             UDEC=np.ascontiguousarray(UDEC[s]),
                             TRI=np.ascontiguousarray(TRI[s])))
    return com, per_core


def build_nc(com, upto=99, dbg=()):
    nc = bacc.Bacc(name="autoformer")
    inp = {k: nc.dram_tensor(k, list(v.shape), F32, kind="ExternalInput")
           for k, v in com.items()}
    UENC = nc.dram_tensor("UENC", [BPC, 23, L], F32, kind="ExternalInput")
    UDEC = nc.dram_tensor("UDEC", [BPC, 23, L], F32, kind="ExternalInput")
    TRI = nc.dram_tensor("TRI", [BPC, CO, L], F32, kind="ExternalInput")
    OUT = nc.dram_tensor("OUT", [BPC, PRED, CO], F32, kind="ExternalOutput")
    ATT = [nc.dram_tensor(f"ATT{l}", [BPC, L, DM], F32, kind="ExternalOutput")
           for l in range(2)]
    dbg_outs = {}

    def dbgt(name, shape):
        if name in dbg:
            h = nc.dram_tensor("DBG_" + name, list(shape), F32,
                               kind="ExternalOutput")
            dbg_outs[name] = h
            return h
        return None

    with tile.TileContext(nc) as tc, ExitStack() as ctx:
        _build_body(nc, tc, ctx, inp, UENC, UDEC, TRI, OUT, ATT, upto, dbgt)
    nc.finalize()
    return nc, dbg_outs


def _build_body(nc, tc, ctx, inp, UENC, UDEC, TRI, OUT, ATT, upto, dbgt):
    cp = ctx.enter_context(tc.tile_pool(name="consts", bufs=1))
    dramp = ctx.enter_context(tc.tile_pool(name="spill", bufs=1, space="DRAM"))
    psp = [None]  # current psum pool, set per phase

    def cload(name, shape, src):
        t = cp.tile(list(shape), F32, name="c_" + name)
        nc.sync.dma_start(t[:], src)
        return t

    fcosA = cload("fcosA", (128, NF), inp["FCOS"][0:128, :])
    fcosB = cload("fcosB", (64, NF), inp["FCOS"][128:L, :])
    fsinA = cload("fsinA", (128, NF), inp["FSIN"][0:128, :])
    fsinB = cload("fsinB", (64, NF), inp["FSIN"][128:L, :])
    icos = cload("icos", (NF, L), inp["ICOS"][:, :])
    isin = cload("isin", (NF, L), inp["ISIN"][:, :])
    adtA = cload("adtA", (128, L), inp["ADT"][0:128, :])
    adtB = cload("adtB", (64, L), inp["ADT"][128:L, :])
    cenA = cload("cenA", (128, L), inp["CEN"][0:128, :])
    cenB = cload("cenB", (64, L), inp["CEN"][128:L, :])
    r0A = cload("r0A", (128, L), inp["R0"][0:128, :])
    r0B = cload("r0B", (64, L), inp["R0"][128:L, :])
    r2A = cload("r2A", (128, L), inp["R2"][0:128, :])
    r2B = cload("r2B", (64, L), inp["R2"][128:L, :])
    id1A = cload("id1A", (128, L), inp["ID192"][0:128, :])
    id1B = cload("id1B", (64, L), inp["ID192"][128:L, :])
    id16 = cload("id16", (16, 16), inp["ID16"][:, :])
    iota = cload("iota", (BPC, L), inp["IOTA"][:, :])
    embE = cload("embE", (23, DM), inp["EMB_E"][:, :])
    embD = cload("embD", (23, DM), inp["EMB_D"][:, :])
    epst = cp.tile([128, 1], F32, name="c_eps")
    nc.vector.memset(epst[:], 1e-5)

    def dtile(name, shape):
        return dramp.tile(list(shape), F32, name=name)

    xtc_e = [dtile(f"xtc_e{i}", (BPC, L, DM)) for i in range(3)]
    xct_e = [dtile(f"xct_e{i}", (BPC, DM, L)) for i in range(3)]
    xtc_d0 = dtile("xtc_d0", (BPC, L, DM))
    xct_d0 = dtile("xct_d0", (BPC, DM, L))
    x2tc_d = dtile("x2tc_d", (BPC, L, DM))
    x2ct_d = dtile("x2ct_d", (BPC, DM, L))
    tacc_d = dtile("tacc_d", (BPC, L, DM))
    vfre_b = dtile("vfre_b", (BPC, NF, DM))
    vfim_b = dtile("vfim_b", (BPC, NF, DM))
    kfcre_b = dtile("kfcre_b", (BPC, NF, DM))
    kfcim_b = dtile("kfcim_b", (BPC, NF, DM))
    vfcre_b = dtile("vfcre_b", (BPC, NF, DM))
    vfcim_b = dtile("vfcim_b", (BPC, NF, DM))

    ev_cnt = [0]

    def evict(dst, src):
        ev_cnt[0] += 1
        if ev_cnt[0] % 2 == 0:
            nc.scalar.copy(dst, src)
        else:
            nc.vector.tensor_copy(dst, src)

    def mm(ps_ap, pairs):
        n = len(pairs)
        for i, (lh, rh) in enumerate(pairs):
            nc.tensor.matmul(ps_ap, lhsT=lh, rhs=rh,
                             start=(i == 0), stop=(i == n - 1))

    _psn = [0]

    def pst(p, f, tag, bufs=None):
        _psn[0] += 1
        return psp[0].tile([p, f], F32, tag=tag, bufs=bufs,
                           name=f"{tag}_{_psn[0]}")

    def load_tc(pool, dram3, j, tag):
        a = pool.tile([128, DM], F32, tag=tag + "a", bufs=2)
        b = pool.tile([64, DM], F32, tag=tag + "b", bufs=2)
        nc.sync.dma_start(a[:], dram3[j, 0:128, :])
        nc.sync.dma_start(b[:], dram3[j, 128:L, :])
        return [a, b]

    def load_ct(pool, dram3, j, tag):
        out = []
        for ci, (cs, cl) in enumerate(CCH):
            t = pool.tile([128, L], F32, tag=f"{tag}{ci}", bufs=2)
            nc.sync.dma_start(t[:], dram3[j, cs:cs + cl, :])
            out.append(t)
        return out

    def store_tc(dram3, j, tcs):
        nc.sync.dma_start(dram3[j, 0:128, :], tcs[0][:])
        nc.sync.dma_start(dram3[j, 128:L, :], tcs[1][:])

    def store_ct(dram3, j, cts):
        for ci, (cs, cl) in enumerate(CCH):
            nc.sync.dma_start(dram3[j, cs:cs + cl, :], cts[ci][:])

    def proj_td(pool, xct, w4, tag):
        out = []
        for (ts, tl) in TCH:
            ps = pst(tl, DM, "tQ", 2)
            mm(ps[0:tl, :], [(xct[ci][:, ts:ts + tl], w4[ci][:])
                             for ci in range(4)])
            sb = pool.tile([tl, DM], F32, tag=f"{tag}{ts}")
            evict(sb[:], ps[0:tl, :])
            out.append(sb)
        return out

    def fft_of(ytd, tag):
        re = pst(NF, DM, "tF", 4)
        mm(re[0:NF, :], [(fcosA[:], ytd[0][:]), (fcosB[:], ytd[1][:])])
        im = pst(NF, DM, "tF", 4)
        mm(im[0:NF, :], [(fsinA[:], ytd[0][:]), (fsinB[:], ytd[1][:])])
        return re, im

    def ln_tc(pool, xtc, tag):
        out = []
        for i, (ts, tl) in enumerate(TCH):
            st = pool.tile([tl, 6], F32, tag=f"{tag}st{i}")
            nc.vector.bn_stats(st[:], xtc[i][:])
            mv2 = pool.tile([tl, 2], F32, tag=f"{tag}mv{i}")
            nc.vector.bn_aggr(mv2[:], st[:])
            rs = pool.tile([tl, 1], F32, tag=f"{tag}rs{i}")
            nc.scalar.activation(rs[:], mv2[:, 1:2], AF.Sqrt,
                                 bias=epst[0:tl, :])
            nc.vector.reciprocal(rs[:], rs[:])
            xh = pool.tile([tl, DM], F32, tag=f"{tag}xh{i}")
            nc.vector.tensor_scalar(xh[:], xtc[i][:], mv2[:, 0:1], rs[:],
                                    op0=OP.subtract, op1=OP.mult)
            out.append(xh)
        return out

    def rho_products(wp, qf, kf, rho_r, rho_i, j, need_p, tag):
        if need_p:
            m1 = wp.tile([NF, DM], F32, tag=tag + "m1")
            nc.gpsimd.tensor_tensor(m1[:], qf[0][:], kf[0][:], op=OP.mult)
            m2 = wp.tile([NF, DM], F32, tag=tag + "m2")
            nc.vector.tensor_tensor(m2[:], qf[1][:], kf[1][:], op=OP.mult)
            rep = wp.tile([NF, DM], F32, tag=tag + "re")
            nc.vector.scalar_tensor_tensor(
                rep[:], m1[:], 1.0, m2[:], op0=OP.mult, op1=OP.add,
                accum_out=rho_r[:, j:j + 1])
            m3 = wp.tile([NF, DM], F32, tag=tag + "m3")
            nc.gpsimd.tensor_tensor(m3[:], qf[1][:], kf[0][:], op=OP.mult)
            m4 = wp.tile([NF, DM], F32, tag=tag + "m4")
            nc.vector.tensor_tensor(m4[:], qf[0][:], kf[1][:], op=OP.mult)
            imp = wp.tile([NF, DM], F32, tag=tag + "im")
            nc.vector.scalar_tensor_tensor(
                imp[:], m3[:], 1.0, m4[:], op0=OP.mult, op1=OP.subtract,
                accum_out=rho_i[:, j:j + 1])
            return rep, imp
        m1 = wp.tile([NF, DM], F32, tag=tag + "m1")
        nc.gpsimd.tensor_tensor(m1[:], qf[0][:], kf[0][:], op=OP.mult)
        m2 = wp.tile([NF, DM], F32, tag=tag + "m2")
        nc.vector.tensor_tensor(m2[:], qf[1][:], kf[1][:], op=OP.mult)
        rep = wp.tile([NF, DM], F32, tag=tag + "re")
        nc.vector.scalar_tensor_tensor(
            rep[:], m1[:], 1.0, m2[:], op0=OP.mult, op1=OP.add,
            accum_out=rho_r[:, j:j + 1])
        m3 = wp.tile([NF, DM], F32, tag=tag + "m3")
        nc.gpsimd.tensor_tensor(m3[:], qf[1][:], kf[0][:], op=OP.mult)
        m4 = wp.tile([NF, DM], F32, tag=tag + "m4")
        nc.vector.tensor_tensor(m4[:], qf[0][:], kf[1][:], op=OP.mult)
        imp = wp.tile([NF, DM], F32, tag=tag + "im")
        nc.vector.scalar_tensor_tensor(
            imp[:], m3[:], 1.0, m4[:], op0=OP.mult, op1=OP.subtract,
            accum_out=rho_i[:, j:j + 1])
        return None, None

    def topk_block(wp, tkp, rho_r, rho_i, tag):
        mvp = pst(BPC, L, "tC", 2)
        mm(mvp[0:BPC, :], [(rho_r[:], icos[:]), (rho_i[:], isin[:])])
        mvs = wp.tile([BPC, L], F32, tag=tag + "mvs")
        nc.vector.tensor_copy(mvs[:], mvp[0:BPC, :])
        tv = wp.tile([BPC, 8], F32, tag=tag + "tv")
        nc.vector.max(tv[:], mvs[:])
        ti = wp.tile([BPC, 8], mybir.dt.uint32, tag=tag + "ti")
        nc.vector.max_index(ti[:], tv[:], mvs[:])
        tif = wp.tile([BPC, 8], F32, tag=tag + "tif")
        nc.vector.tensor_copy(tif[:], ti[:])
        negb = wp.tile([BPC, 1], F32, tag=tag + "negb")
        nc.vector.tensor_scalar_mul(negb[:], tv[:, 0:1], -1.0 / DM)
        ex = wp.tile([BPC, TOPK], F32, tag=tag + "ex")
        nc.scalar.activation(ex[:], tv[:, 0:TOPK], AF.Exp,
                             bias=negb[:], scale=1.0 / DM)
        ssum = wp.tile([BPC, 1], F32, tag=tag + "ss")
        nc.vector.tensor_reduce(ssum[:], ex[:], axis=AX.X, op=OP.add)
        nc.vector.reciprocal(ssum[:], ssum[:])
        w5 = wp.tile([BPC, TOPK], F32, tag=tag + "w5")
        nc.vector.tensor_scalar_mul(w5[:], ex[:], ssum[:])
        u = wp.tile([BPC, L], F32, tag=tag + "u")
        nc.vector.tensor_scalar(u[:], iota[:], tif[:, 0:1], w5[:, 0:1],
                                op0=OP.is_equal, op1=OP.mult)
        for k in range(1, TOPK):
            uk = wp.tile([BPC, L], F32, tag=tag + "uk")
            nc.vector.tensor_scalar(uk[:], iota[:], tif[:, k:k + 1],
                                    w5[:, k:k + 1],
                                    op0=OP.is_equal, op1=OP.mult)
            nc.vector.tensor_tensor(u[:], u[:], uk[:], op=OP.add)
        utp_a = pst(128, BPC, "tC", 2)
        mm(utp_a[0:128, :], [(u[:, 0:128], id16[:])])
        utp_b = pst(64, BPC, "tC", 2)
        mm(utp_b[0:64, :], [(u[:, 128:L], id16[:])])
        uta = wp.tile([128, BPC], F32, tag=tag + "usa")
        evict(uta[:], utp_a[0:128, :])
        utb = wp.tile([64, BPC], F32, tag=tag + "usb")
        evict(utb[:], utp_b[0:64, :])
        ufr_p = pst(NF, BPC, "tC", 2)
        mm(ufr_p[0:NF, :], [(fcosA[:], uta[:]), (fcosB[:], utb[:])])
        ufi_p = pst(NF, BPC, "tC", 2)
        mm(ufi_p[0:NF, :], [(fsinA[:], uta[:]), (fsinB[:], utb[:])])
        ufr = tkp.tile([NF, BPC], F32, tag=tag + "ufrs", bufs=1)
        evict(ufr[:], ufr_p[0:NF, :])
        ufi = tkp.tile([NF, BPC], F32, tag=tag + "ufis", bufs=1)
        evict(ufi[:], ufi_p[0:NF, :])
        return ufr, ufi, u

    def agg_ct(wp, vfre, vfim, ufr, ufi, j, tag):
        tmp1 = wp.tile([NF, L], F32, tag=tag + "t1")
        nc.vector.tensor_scalar_mul(tmp1[:], isin[:], ufi[:, j:j + 1])
        g1 = wp.tile([NF, L], F32, tag=tag + "g1")
        nc.vector.scalar_tensor_tensor(g1[:], icos[:], ufr[:, j:j + 1],
                                       tmp1[:], op0=OP.mult, op1=OP.subtract)
        tmp2 = wp.tile([NF, L], F32, tag=tag + "t2")
        nc.gpsimd.tensor_scalar_mul(tmp2[:], isin[:], ufr[:, j:j + 1])
        g2 = wp.tile([NF, L], F32, tag=tag + "g2")
        nc.vector.scalar_tensor_tensor(g2[:], icos[:], ufi[:, j:j + 1],
                                       tmp2[:], op0=OP.mult, op1=OP.add)
        outs = []
        for ci, (cs, cl) in enumerate(CCH):
            ps = pst(128, L, "tAG", 2)
            mm(ps[0:128, :], [(vfre[:, cs:cs + cl], g1[:]),
                              (vfim[:, cs:cs + cl], g2[:])])
            sb = wp.tile([128, L], F32, tag=f"{tag}as{ci}")
            evict(sb[:], ps[0:128, :])
            outs.append(sb)
        return outs

    def proj_from_ct(src_ct, w4, tag):
        outp = []
        for (ts, tl) in TCH:
            ps = pst(tl, DM, "tAO", 1)
            mm(ps[0:tl, :], [(src_ct[ci][:, ts:ts + tl], w4[ci][:])
                             for ci in range(4)])
            outp.append(ps)
        return outp

    def decomp_dual(wp, u_tc, tag, want_ct=True):
        tc_out = []
        for i, (ts, tl) in enumerate(TCH):
            ps = pst(tl, DM, "tD", 2)
            mm(ps[0:tl, :], [(adtA[:, ts:ts + tl], u_tc[0][:]),
                             (adtB[:, ts:ts + tl], u_tc[1][:])])
            sb = wp.tile([tl, DM], F32, tag=tag + f"tcs{i}")
            evict(sb[:], ps[0:tl, :])
            tc_out.append(sb)
        ct_out = None
        if want_ct:
            ct_out = []
            for ci, (cs, cl) in enumerate(CCH):
                ps = pst(128, L, "tD", 2)
                mm(ps[0:128, :], [(u_tc[0][:, cs:cs + cl], adtA[:]),
                                  (u_tc[1][:, cs:cs + cl], adtB[:])])
                sb = wp.tile([128, L], F32, tag=tag + f"cts{ci}")
                evict(sb[:], ps[0:128, :])
                ct_out.append(sb)
        return tc_out, ct_out

    def load_w4(pool, name, tag):
        w = []
        for ci, (cs, cl) in enumerate(CCH):
            t = pool.tile([128, DM], F32, tag=f"{tag}{ci}", bufs=1)
            nc.sync.dma_start(t[:], inp[name][cs:cs + cl, :])
            w.append(t)
        return w

    # ======================= STAGE 0: embeddings =======================
    d_x0e = dbgt("x0e", (BPC, L, DM))
    d_x0d = dbgt("x0d", (BPC, L, DM))
    with tc.tile_pool(name="s0", bufs=3) as wp, \
            tc.tile_pool(name="s0p", bufs=1, space="PSUM") as pp0:
        psp[0] = pp0
        for j in range(BPC):
            for (udram, xtcb, xctb, emb, dd) in (
                    (UENC, xtc_e[0], xct_e[0], embE, d_x0e),
                    (UDEC, xtc_d0, xct_d0, embD, d_x0d)):
                ue = wp.tile([23, L], F32, tag="ue")
                nc.sync.dma_start(ue[:], udram[j, :, :])
                tcs = []
                for i, (ts, tl) in enumerate(TCH):
                    ps = pst(tl, DM, "tQ", 2)
                    mm(ps[0:tl, :], [(ue[:, ts:ts + tl], emb[:])])
                    sb = wp.tile([tl, DM], F32, tag=f"embs{i}")
                    evict(sb[:], ps[0:tl, :])
                    tcs.append(sb)
                store_tc(xtcb, j, tcs)
                if dd is not None:
                    store_tc(dd, j, tcs)
                cts = []
                for ci, (cs, cl) in enumerate(CCH):
                    ps = pst(128, L, "tC", 2)
                    mm(ps[0:128, :], [(emb[:, cs:cs + cl], ue[:])])
                    sb = wp.tile([128, L], F32, tag=f"embc{ci}")
                    evict(sb[:], ps[0:128, :])
                    cts.append(sb)
                store_ct(xctb, j, cts)
    if upto < 1:
        return

    # ======================= ENCODER =======================
    def ac_loop_a(wp, xct_src, wq, wk, wv, rho_r, rho_i, j,
                  corr_dst, vf_spill, tag):
        xct = load_ct(wp, xct_src, j, tag + "x")
        q = proj_td(wp, xct, wq, tag + "q")
        k = proj_td(wp, xct, wk, tag + "k")
        v = proj_td(wp, xct, wv, tag + "v")
        qf_p = fft_of(q, tag + "qf")
        kf_p = fft_of(k, tag + "kf")
        vf_p = fft_of(v, tag + "vf")
        qf, kf = [], []
        for nm, p, lst in (("qr", qf_p[0], qf), ("qi", qf_p[1], qf),
                           ("kr", kf_p[0], kf), ("ki", kf_p[1], kf)):
            sb = wp.tile([NF, DM], F32, tag=tag + nm)
            evict(sb[:], p[0:NF, :])
            lst.append(sb)
        for nm, p, dst in (("vr", vf_p[0], vf_spill[0]),
                           ("vi", vf_p[1], vf_spill[1])):
            sb = wp.tile([NF, DM], F32, tag=tag + nm)
            evict(sb[:], p[0:NF, :])
            nc.sync.dma_start(dst[j, :, :], sb[:])
        rep, imp = rho_products(wp, qf, kf, rho_r, rho_i, j,
                                corr_dst is not None, tag + "p")
        if corr_dst is not None:
            for (ts, tl) in TCH:
                ps = pst(tl, DM, "tC", 2)
                mm(ps[0:tl, :], [(icos[:, ts:ts + tl], rep[:]),
                                 (isin[:, ts:ts + tl], imp[:])])
                sb = wp.tile([tl, DM], F32, tag=tag + f"cos{ts}")
                evict(sb[:], ps[0:tl, :])
                nc.sync.dma_start(corr_dst[j, ts:ts + tl, :], sb[:])

    def ffn_block(wp, w1, w2, x_ct, x_tc_res, tagp):
        h = []
        for di in range(16):
            ps = pst(128, L, "tH", 2)
            mm(ps[0:128, :],
               [(w1[ci][:, di * 128:(di + 1) * 128], x_ct[ci][:])
                for ci in range(4)])
            hs = wp.tile([128, L], F32, tag=f"{tagp}hs{di}")
            nc.scalar.activation(hs[:], ps[0:128, :], AF.Gelu)
            h.append(hs)
        u3 = []
        for i, (ts, tl) in enumerate(TCH):
            ps = pst(tl, DM, "tY", 1)
            mm(ps[0:tl, :], [(h[di][:, ts:ts + tl], w2[di][:])
                             for di in range(16)])
            sb = wp.tile([tl, DM], F32, tag=f"{tagp}u3{i}")
            nc.vector.tensor_tensor(sb[:], ps[0:tl, :], x_tc_res[i][:],
                                    op=OP.add)
            u3.append(sb)
        return u3

    def load_ffn_w(wp, w1name, w2name, tagp):
        w1 = []
        for ci, (cs, cl) in enumerate(CCH):
            t = wp.tile([128, DFF], F32, tag=f"{tagp}w1{ci}", bufs=1)
            nc.sync.dma_start(t[:], inp[w1name][cs:cs + cl, :])
            w1.append(t)
        w2 = []
        for di in range(16):
            t = wp.tile([128, DM], F32, tag=f"{tagp}w2{di}", bufs=1)
            nc.sync.dma_start(t[:], inp[w2name][di * 128:(di + 1) * 128, :])
            w2.append(t)
        return w1, w2

    def enc_layer(l, xin_tc, xin_ct, xout_tc, xout_ct, att_dst):
        d_x = dbgt(f"x{l + 1}e", (BPC, L, DM))
        d_u = dbgt(f"u{l}e", (BPC, L)) if l == 0 else None
        with ExitStack() as ls:
            tkp = ls.enter_context(tc.tile_pool(name=f"e{l}tk", bufs=1))
            with ExitStack() as lsa:
                wp = lsa.enter_context(tc.tile_pool(name=f"e{l}a", bufs=2))
                psp[0] = lsa.enter_context(
                    tc.tile_pool(name=f"e{l}ap", bufs=1, space="PSUM"))
                wq = load_w4(wp, f"e{l}_wq", "wwq")
                wk = load_w4(wp, f"e{l}_wk", "wwk")
                wv = load_w4(wp, f"e{l}_wv", "wwv")
                rho_r = tkp.tile([NF, BPC], F32, tag="rhor", bufs=1)
                rho_i = tkp.tile([NF, BPC], F32, tag="rhoi", bufs=1)
                for j in range(BPC):
                    ac_loop_a(wp, xin_ct, wq, wk, wv, rho_r, rho_i, j,
                              att_dst, (vfre_b, vfim_b), "A")
                ufr, ufi, u_dbg = topk_block(wp, tkp, rho_r, rho_i, f"e{l}tk")
                if d_u is not None:
                    nc.sync.dma_start(d_u[:, :], u_dbg[:])
            wp = ls.enter_context(tc.tile_pool(name=f"e{l}b", bufs=1))
            psp[0] = ls.enter_context(
                tc.tile_pool(name=f"e{l}bp", bufs=1, space="PSUM"))
            wo = load_w4(wp, f"e{l}_wo", "wwo")
            w1, w2 = load_ffn_w(wp, f"e{l}_w1", f"e{l}_w2", "B")
            for j in range(BPC):
                x_tc = load_tc(wp, xin_tc, j, "Bx")
                vfre = wp.tile([NF, DM], F32, tag="Bvr", bufs=2)
                nc.sync.dma_start(vfre[:], vfre_b[j, :, :])
                vfim = wp.tile([NF, DM], F32, tag="Bvi", bufs=2)
                nc.sync.dma_start(vfim[:], vfim_b[j, :, :])
                agg = agg_ct(wp, vfre, vfim, ufr, ufi, j, "Bg")
                ao_p = proj_from_ct(agg, wo, "Bao")
                u1 = []
                for i, (ts, tl) in enumerate(TCH):
                    sb = wp.tile([tl, DM], F32, tag=f"Bu1{i}")
                    nc.vector.tensor_tensor(sb[:], ao_p[i][0:tl, :],
                                            x_tc[i][:], op=OP.add)
                    u1.append(sb)
                x2_tc, x2_ct = decomp_dual(wp, u1, "Bd1")
                u3 = ffn_block(wp, w1, w2, x2_ct, x2_tc, "B")
                x3_tc, x3_ct = decomp_dual(wp, u3, "Bd2")
                store_tc(xout_tc, j, x3_tc)
                store_ct(xout_ct, j, x3_ct)
                if d_x is not None:
                    store_tc(d_x, j, x3_tc)

    enc_layer(0, xtc_e[0], xct_e[0], xtc_e[1], xct_e[1], ATT[0])
    if upto < 2:
        return
    enc_layer(1, xtc_e[1], xct_e[1], xtc_e[2], xct_e[2], ATT[1])
    if upto < 3:
        return

    # ================ STAGE 3: enc_norm + cross K/V FFTs ================
    d_kfc = dbgt("kfc", (BPC, NF, DM))
    with tc.tile_pool(name="s3", bufs=2) as wp, \
            tc.tile_pool(name="s3p", bufs=1, space="PSUM") as pp3:
        psp[0] = pp3
        wkc = load_w4(wp, "dc_wk", "wkc")
        wvc = load_w4(wp, "dc_wv", "wvc")
        for j in range(BPC):
            x_tc = load_tc(wp, xtc_e[2], j, "Nx")
            xh = ln_tc(wp, x_tc, "Nln")
            cen_ct = []
            for ci, (cs, cl) in enumerate(CCH):
                ps = pst(128, L, "tC", 2)
                mm(ps[0:128, :], [(xh[0][:, cs:cs + cl], cenA[:]),
                                  (xh[1][:, cs:cs + cl], cenB[:])])
                sb = wp.tile([128, L], F32, tag=f"Ncs{ci}")
                evict(sb[:], ps[0:128, :])
                cen_ct.append(sb)
            for w4, spill in ((wkc, (kfcre_b, kfcim_b)),
                              (wvc, (vfcre_b, vfcim_b))):
                y = proj_td(wp, cen_ct, w4, "Nkv")
                re_p, im_p = fft_of(y, "Nf")
                for p, dst, nm in ((re_p, spill[0], "r"), (im_p, spill[1], "i")):
                    sb = wp.tile([NF, DM], F32, tag="Nfs" + nm)
                    evict(sb[:], p[0:NF, :])
                    nc.sync.dma_start(dst[j, :, :], sb[:])
                    if nm == "r" and spill[0] is kfcre_b and d_kfc is not None:
                        nc.sync.dma_start(d_kfc[j, :, :], sb[:])
    if upto < 4:
        return

    # ================ STAGE 4: decoder self-attention ================
    d_x2 = dbgt("x2d", (BPC, L, DM))
    with ExitStack() as ls4:
        tkp = ls4.enter_context(tc.tile_pool(name="s4tk", bufs=1))
        with ExitStack() as ls4a:
            wp = ls4a.enter_context(tc.tile_pool(name="s4a", bufs=2))
            psp[0] = ls4a.enter_context(
                tc.tile_pool(name="s4ap", bufs=1, space="PSUM"))
            wq = load_w4(wp, "ds_wq", "dwq")
            wk = load_w4(wp, "ds_wk", "dwk")
            wv = load_w4(wp, "ds_wv", "dwv")
            rho_r = tkp.tile([NF, BPC], F32, tag="rhor2", bufs=1)
            rho_i = tkp.tile([NF, BPC], F32, tag="rhoi2", bufs=1)
            for j in range(BPC):
                ac_loop_a(wp, xct_d0, wq, wk, wv, rho_r, rho_i, j,
                          None, (vfre_b, vfim_b), "D")
            ufr, ufi, _ = topk_block(wp, tkp, rho_r, rho_i, "dtk")
        wp = ls4.enter_context(tc.tile_pool(name="s4b", bufs=1))
        psp[0] = ls4.enter_context(
            tc.tile_pool(name="s4bp", bufs=1, space="PSUM"))
        wo = load_w4(wp, "ds_wo", "dwo")
        for j in range(BPC):
            x_tc = load_tc(wp, xtc_d0, j, "Dx")
            vfre = wp.tile([NF, DM], F32, tag="Dvr", bufs=2)
            nc.sync.dma_start(vfre[:], vfre_b[j, :, :])
            vfim = wp.tile([NF, DM], F32, tag="Dvi", bufs=2)
            nc.sync.dma_start(vfim[:], vfim_b[j, :, :])
            agg = agg_ct(wp, vfre, vfim, ufr, ufi, j, "Dg")
            ao_p = proj_from_ct(agg, wo, "Dao")
            u1 = []
            for i, (ts, tl) in enumerate(TCH):
                sb = wp.tile([tl, DM], F32, tag=f"Du1{i}")
                nc.vector.tensor_tensor(sb[:], ao_p[i][0:tl, :],
                                        x_tc[i][:], op=OP.add)
                u1.append(sb)
            x2_tc, x2_ct = decomp_dual(wp, u1, "Dd1")
            tacc = []
            for i in range(2):
                sb = wp.tile([TCH[i][1], DM], F32, tag=f"Dt{i}")
                nc.vector.tensor_tensor(sb[:], u1[i][:], x2_tc[i][:],
                                        op=OP.subtract)
                tacc.append(sb)
            store_tc(x2tc_d, j, x2_tc)
            store_ct(x2ct_d, j, x2_ct)
            store_tc(tacc_d, j, tacc)
            if d_x2 is not None:
                store_tc(d_x2, j, x2_tc)
    if upto < 5:
        return

    # ================ STAGE 5/6/7: cross attn + FFN + head ================
    with ExitStack() as ls5:
        tkp = ls5.enter_context(tc.tile_pool(name="s5tk", bufs=1))
        ls5a = ExitStack()
        wp = ls5a.enter_context(tc.tile_pool(name="s5a", bufs=2))
        psp[0] = ls5a.enter_context(
            tc.tile_pool(name="s5ap", bufs=1, space="PSUM"))
        cwq = load_w4(wp, "dc_wq", "cwq")
        rho_r = tkp.tile([NF, BPC], F32, tag="rhor3", bufs=1)
        rho_i = tkp.tile([NF, BPC], F32, tag="rhoi3", bufs=1)
        for j in range(BPC):
            xct = load_ct(wp, x2ct_d, j, "Cx")
            q = proj_td(wp, xct, cwq, "Cq")
            qf_p = fft_of(q, "Cqf")
            qf = []
            for nm, p in (("qr", qf_p[0]), ("qi", qf_p[1])):
                sb = wp.tile([NF, DM], F32, tag="C" + nm)
                evict(sb[:], p[0:NF, :])
                qf.append(sb)
            kfr = wp.tile([NF, DM], F32, tag="Ckr", bufs=2)
            nc.sync.dma_start(kfr[:], kfcre_b[j, :, :])
            kfi = wp.tile([NF, DM], F32, tag="Cki", bufs=2)
            nc.sync.dma_start(kfi[:], kfcim_b[j, :, :])
            rho_products(wp, qf, (kfr, kfi), rho_r, rho_i, j, False, "Cp")
        ufr3, ufi3, _ = topk_block(wp, tkp, rho_r, rho_i, "ctk")
        ls5a.close()
        wp = ls5.enter_context(tc.tile_pool(name="s5b", bufs=1))
        psp[0] = ls5.enter_context(
            tc.tile_pool(name="s5bp", bufs=1, space="PSUM"))
        cwo = load_w4(wp, "dc_wo", "cwo")
        w1, w2 = load_ffn_w(wp, "d_w1", "d_w2", "F")
        wtj = []
        for jj in range(3):
            per = []
            for ci, (cs, cl) in enumerate(CCH):
                t = wp.tile([128, CO], F32, tag=f"dwt{jj}{ci}", bufs=1)
                nc.sync.dma_start(t[:], inp["d_wt"][jj, cs:cs + cl, :])
                per.append(t)
            wtj.append(per)
        wpp = []
        for ci, (cs, cl) in enumerate(CCH):
            t = wp.tile([128, CO], F32, tag=f"wpp{ci}", bufs=1)
            nc.sync.dma_start(t[:], inp["WPP"][cs:cs + cl, :])
            wpp.append(t)

        if upto < 6:
            return

        shift_consts = ((r0A, r0B), (id1A, id1B), (r2A, r2B))
        for j in range(BPC):
            x2_tc = load_tc(wp, x2tc_d, j, "Fx2")
            tacc = load_tc(wp, tacc_d, j, "Fta")
            vfre = wp.tile([NF, DM], F32, tag="Fvr", bufs=2)
            nc.sync.dma_start(vfre[:], vfcre_b[j, :, :])
            vfim = wp.tile([NF, DM], F32, tag="Fvi", bufs=2)
            nc.sync.dma_start(vfim[:], vfcim_b[j, :, :])
            agg = agg_ct(wp, vfre, vfim, ufr3, ufi3, j, "Fg")
            ca_p = proj_from_ct(agg, cwo, "Fca")
            u2 = []
            for i, (ts, tl) in enumerate(TCH):
                sb = wp.tile([tl, DM], F32, tag=f"Fu2{i}")
                nc.vector.tensor_tensor(sb[:], ca_p[i][0:tl, :],
                                        x2_tc[i][:], op=OP.add)
                u2.append(sb)
            x3_tc, x3_ct = decomp_dual(wp, u2, "Fd2")
            for i in range(2):
                tmp = wp.tile([TCH[i][1], DM], F32, tag=f"Ftm{i}")
                nc.vector.tensor_tensor(tmp[:], u2[i][:], x3_tc[i][:],
                                        op=OP.subtract)
                nc.vector.tensor_tensor(tacc[i][:], tacc[i][:], tmp[:],
                                        op=OP.add)
            u3 = ffn_block(wp, w1, w2, x3_ct, x3_tc, "F")
            x4_tc, _ = decomp_dual(wp, u3, "Fd3", want_ct=False)
            for i in range(2):
                tmp = wp.tile([TCH[i][1], DM], F32, tag=f"Ftn{i}")
                nc.vector.tensor_tensor(tmp[:], u3[i][:], x4_tc[i][:],
                                        op=OP.subtract)
                nc.vector.tensor_tensor(tacc[i][:], tacc[i][:], tmp[:],
                                        op=OP.add)
            xh = ln_tc(wp, x4_tc, "Fln")
            cen_ct = []
            for ci, (cs, cl) in enumerate(CCH):
                ps = pst(128, L, "tD", 2)
                mm(ps[0:128, :], [(xh[0][:, cs:cs + cl], cenA[:]),
                                  (xh[1][:, cs:cs + cl], cenB[:])])
                sb = wp.tile([128, L], F32, tag=f"Fcs{ci}")
                evict(sb[:], ps[0:128, :])
                cen_ct.append(sb)
            seas_p = pst(CO, L, "tY", 1)
            mm(seas_p[0:CO, :], [(wpp[ci][:], cen_ct[ci][:])
                                 for ci in range(4)])
            sj_all = []
            for jj in range(3):
                rA, rB = shift_consts[jj]
                sjc = []
                for ci, (cs, cl) in enumerate(CCH):
                    ps = pst(128, L, "tAG", 2)
                    mm(ps[0:128, :], [(tacc[0][:, cs:cs + cl], rA[:]),
                                      (tacc[1][:, cs:cs + cl], rB[:])])
                    sb = wp.tile([128, L], F32, tag=f"Fss{jj}{ci}")
                    evict(sb[:], ps[0:128, :])
                    sjc.append(sb)
                sj_all.append(sjc)
            res_p = pst(CO, L, "tH", 2)
            mm(res_p[0:CO, :], [(wtj[jj][ci][:], sj_all[jj][ci][:])
                                for jj in range(3) for ci in range(4)])
            tri = wp.tile([CO, L], F32, tag="Ftri")
            nc.sync.dma_start(tri[:], TRI[j, :, :])
            o1 = wp.tile([CO, L], F32, tag="Fo1")
            nc.vector.tensor_tensor(o1[:], seas_p[0:CO, :], tri[:], op=OP.add)
            o2 = wp.tile([CO, L], F32, tag="Fo2")
            nc.vector.tensor_tensor(o2[:], res_p[0:CO, :], o1[:], op=OP.add)
            nc.sync.dma_start(OUT[j, :, :].rearrange("t c -> c t"),
                              o2[:, LBL:L])


_CACHE = {}


def _build_runner(com):
    """Compile once; return a callable in_maps -> per-core outputs, plus a
    raw executor handle for timing."""
    import jax
    from jax.sharding import Mesh, PartitionSpec, NamedSharding
    from jax.experimental.shard_map import shard_map
    import concourse.bass2jax as b2j

    nc, _ = build_nc(com)
    b2j.install_neuronx_cc_hook()
    partition_name = (nc.partition_id_tensor.name
                      if nc.partition_id_tensor else None)
    in_names, out_names, out_avals, zero_outs = [], [], [], []
    for alloc in nc.m.functions[0].allocations:
        if not isinstance(alloc, mybir.MemoryLocationSet):
            continue
        name = alloc.memorylocations[0].name
        if alloc.kind == "ExternalInput":
            if name != partition_name:
                in_names.append(name)
        elif alloc.kind == "ExternalOutput":
            shape = tuple(alloc.tensor_shape)
            dtype = mybir.dt.np(alloc.dtype)
            out_names.append(name)
            out_avals.append(jax.core.ShapedArray(shape, dtype))
            zero_outs.append(np.zeros(shape, dtype))
    all_in = list(in_names) + list(out_names)
    if partition_name is not None:
        all_in.append(partition_name)

    def _body(*args):
        operands = list(args)
        if partition_name is not None:
            operands.append(b2j.partition_id_tensor())
        return tuple(b2j._bass_exec_p.bind(
            *operands, out_avals=tuple(out_avals),
            in_names=tuple(all_in), out_names=tuple(out_names),
            lowering_input_output_aliases=(), sim_require_finite=True,
            sim_require_nnan=True, nc=nc))

    devices = jax.devices()[:NCORES]
    mesh = Mesh(np.asarray(devices), ("core",))
    nin = len(in_names) + len(out_names)
    fn = jax.jit(shard_map(_body, mesh=mesh,
                           in_specs=(PartitionSpec("core"),) * nin,
                           out_specs=(PartitionSpec("core"),) * len(out_names),
                           check_rep=False), keep_unused=True)
    sh = NamedSharding(mesh, PartitionSpec("core"))
    return dict(fn=fn, in_names=in_names, out_names=out_names,
                zero_outs=zero_outs, sh=sh)


def _run(com, per_core):
    import jax
    if "runner" not in _CACHE:
        _CACHE["runner"] = _build_runner(com)
    R = _CACHE["runner"]
    in_maps = []
    for c in range(NCORES):
        m = dict(com)
        m.update(per_core[c])
        in_maps.append(m)
    concat_in = [np.concatenate([np.asarray(in_maps[c][nm])
                                 for c in range(NCORES)], axis=0)
                 for nm in R["in_names"]]
    concat_zero = [np.zeros((NCORES * z.shape[0], *z.shape[1:]), z.dtype)
                   for z in R["zero_outs"]]
    dev_in = [jax.device_put(a, R["sh"]) for a in concat_in + concat_zero]
    _CACHE["dev_in"] = dev_in
    outs = R["fn"](*dev_in)
    jax.block_until_ready(outs)
    res = {nm: np.asarray(o) for nm, o in zip(R["out_names"], outs)}
    return res


def kernel(x_enc, x_mark_enc, x_dec, x_mark_dec, params):
    hc = _host_consts()
    com, per_core = _prep_host(x_enc, x_mark_enc, x_dec, x_mark_dec, params, hc)
    res = _run(com, per_core)
    out = res["OUT"].reshape(NCORES * BPC, PRED, CO)
    att0 = res["ATT0"].reshape(NCORES * BPC, L, DM)
    att1 = res["ATT1"].reshape(NCORES * BPC, L, DM)
    B = out.shape[0]
    return (out.astype(np.float32),
            att0.reshape(B, L, 8, 64).astype(np.float32),
            att1.reshape(B, L, 8, 64).astype(np.float32))


def bench_exec_ns(reps=20):
    """Amortized per-execution time of the cached runner (call kernel() first)."""
    import jax, time as _t
    R = _CACHE["runner"]
    dev_in = _CACHE["dev_in"]
    for _ in range(3):
        outs = R["fn"](*dev_in)
    jax.block_until_ready(outs)
    t0 = _t.time()
    allouts = [R["fn"](*dev_in) for _ in range(reps)]
    jax.block_until_ready(allouts)
    return (_t.time() - t0) / reps * 1e9


# revision 21
# speedup vs baseline: 1.0898x; 1.0898x over previous
"""Autoformer forward pass as a Bass/Tile kernel for Trainium2, 8 NeuronCores,
pure data parallel over batch (B=128 -> 16 samples/core).

Host side precomputes constant matrices (DFT / moving-average / centering /
shift permutations), folds layernorm gammas into downstream weights, and
builds the tiny embedding unfolds. The device kernel does everything else in
fp32: projections, FFT autocorrelation via DFT matmuls, batched top-5 delay
selection (vector.max/max_index), delay aggregation via irfft(Vf*conj(Uf))
matmuls, series decomposition via (I-A) matmuls, FFN with exact erf Gelu.

Layouts per sample:
  tc (192 x C): tiles (128,C)+(64,C), time on partitions
  ct (512 x T): 4 tiles (128,T), channels on partitions
  fd (97 x C):  one tile, rfft bins on partitions
matmul: out[m,n] = sum_k lhsT[k,m] rhs[k,n], k on partitions.
"""
import sys, os
sys.path.insert(0, "/opt/trn_rl_repo")
os.environ.setdefault("MYCRO_LOCAL_CACHE", "1")

import numpy as np
from contextlib import ExitStack

import concourse.bass as bass
import concourse.bacc as bacc
import concourse.tile as tile
from concourse import mybir
from concourse.bass_utils import run_bass_kernel_spmd

F32 = mybir.dt.float32
AF = mybir.ActivationFunctionType
OP = mybir.AluOpType
AX = mybir.AxisListType

L, NF, DM, DFF, CO, NM = 192, 97, 512, 2048, 6, 5
PRED, LBL, MA, TOPK = 96, 96, 25, 5
NCORES, BPC = 8, 16
TCH = [(0, 128), (128, 64)]
CCH = [(0, 128), (128, 128), (256, 128), (384, 128)]


def _host_consts():
    t = np.arange(L)[:, None].astype(np.float64)
    f = np.arange(NF)[None, :].astype(np.float64)
    ang = 2.0 * np.pi * t * f / L
    FCOS = np.cos(ang).astype(np.float32)
    FSIN = (-np.sin(ang)).astype(np.float32)
    wgt = np.full((NF, 1), 2.0); wgt[0, 0] = 1.0; wgt[NF - 1, 0] = 1.0
    ang2 = 2.0 * np.pi * np.arange(NF)[:, None] * np.arange(L)[None, :] / L
    ICOS = (wgt / L * np.cos(ang2)).astype(np.float32)
    ISIN = (-wgt / L * np.sin(ang2)).astype(np.float32)
    A = np.zeros((L, L), dtype=np.float64)
    pad = (MA - 1) // 2
    for tt in range(L):
        for i in range(MA):
            s = min(max(tt + i - pad, 0), L - 1)
            A[tt, s] += 1.0 / MA
    A = A.astype(np.float32)
    ADT = (np.eye(L, dtype=np.float32) - A).T.copy()
    CEN = (np.eye(L, dtype=np.float32) - np.float32(1.0 / L)).astype(np.float32)
    R = []
    for j in range(3):
        Rj = np.zeros((L, L), dtype=np.float32)
        for tt in range(L):
            Rj[(tt + j - 1) % L, tt] = 1.0
        R.append(Rj)
    return dict(FCOS=FCOS, FSIN=FSIN, ICOS=ICOS, ISIN=ISIN, A=A, ADT=ADT,
                CEN=CEN, R0=R[0], R2=R[2], ID192=np.eye(L, dtype=np.float32),
                ID16=np.eye(16, dtype=np.float32),
                IOTA=np.tile(np.arange(L, dtype=np.float32), (BPC, 1)))


def _prep_host(x_enc, x_mark_enc, x_dec, x_mark_dec, params, hc):
    B = np.asarray(x_enc).shape[0]
    A = hc["A"]
    xe = np.asarray(x_enc, np.float32)
    mean_xe = xe.mean(axis=1, keepdims=True, dtype=np.float32)
    trend = np.einsum("ts,bsc->btc", A, xe).astype(np.float32)
    seasonal = (xe - trend).astype(np.float32)
    trend_init = np.concatenate(
        [trend[:, L - LBL:], np.broadcast_to(mean_xe, (B, PRED, CO))], axis=1)
    dec_in = np.concatenate(
        [seasonal[:, L - LBL:], np.zeros((B, PRED, CO), np.float32)], axis=1)

    def unfold(x, mark):
        xT = np.transpose(x, (0, 2, 1))
        rows = [np.roll(xT, 1 - j, axis=2) for j in range(3)]
        return np.ascontiguousarray(
            np.concatenate(rows + [np.transpose(mark, (0, 2, 1))], axis=1),
            np.float32)

    UENC = unfold(xe, np.asarray(x_mark_enc, np.float32))
    UDEC = unfold(dec_in, np.asarray(x_mark_dec, np.float32))
    TRI = np.ascontiguousarray(np.transpose(trend_init, (0, 2, 1)), np.float32)

    p = params
    g_enc = np.asarray(p["enc_norm"]["g"], np.float32)
    g_dec = np.asarray(p["dec_norm"]["g"], np.float32)

    com = {k: hc[k] for k in ("FCOS", "FSIN", "ICOS", "ISIN", "ADT", "CEN",
                              "R0", "R2", "ID192", "ID16", "IOTA")}

    def emb(pp):
        Wc = np.asarray(pp["Wc"], np.float32).reshape(18, DM)
        Wt = np.asarray(pp["Wt"], np.float32)
        return np.ascontiguousarray(np.concatenate([Wc, Wt], axis=0), np.float32)

    com["EMB_E"] = emb(p["enc_emb"])
    com["EMB_D"] = emb(p["dec_emb"])

    def attnw(ap, pre=None):
        d = {}
        for nm in ("Wq", "Wk", "Wv", "Wo"):
            w = np.asarray(ap[nm], np.float32)
            if pre is not None and nm in ("Wk", "Wv"):
                w = (pre[:, None] * w).astype(np.float32)
            d[nm.lower()] = np.ascontiguousarray(w)
        for nm in ("bq", "bk", "bv", "bo"):
            b = np.asarray(ap[nm], np.float32)
            assert not np.any(b), f"nonzero bias {nm} unsupported"
        return d

    for l in range(2):
        el = p["enc_layers"][l]
        for k, v in attnw(el["attn"]).items():
            com[f"e{l}_{k}"] = v
        com[f"e{l}_w1"] = np.ascontiguousarray(np.asarray(el["W1"], np.float32))
        com[f"e{l}_w2"] = np.ascontiguousarray(np.asarray(el["W2"], np.float32))
    dl = p["dec_layers"][0]
    for k, v in attnw(dl["self"]).items():
        com[f"ds_{k}"] = v
    for k, v in attnw(dl["cross"], pre=g_enc).items():
        com[f"dc_{k}"] = v
    com["d_w1"] = np.ascontiguousarray(np.asarray(dl["W1"], np.float32))
    com["d_w2"] = np.ascontiguousarray(np.asarray(dl["W2"], np.float32))
    com["d_wt"] = np.ascontiguousarray(np.asarray(dl["Wt"], np.float32))
    com["WPP"] = np.ascontiguousarray(
        (g_dec[:, None] * np.asarray(p["Wp"], np.float32)).astype(np.float32))
    assert not np.any(np.asarray(p["bp"], np.float32)), "nonzero bp unsupported"
    # enc_norm/dec_norm biases cancel mathematically under time-centering

    per_core = []
    for c in range(NCORES):
        s = slice(c * BPC, (c + 1) * BPC)
        per_core.append(dict(UENC=np.ascontiguousarray(UENC[s]),
                             UDEC=np.ascontiguousarray(UDEC[s]),
                             TRI=np.ascontiguousarray(TRI[s])))
    return com, per_core


def build_nc(com, upto=99, dbg=()):
    nc = bacc.Bacc(name="autoformer")
    inp = {k: nc.dram_tensor(k, list(v.shape), F32, kind="ExternalInput")
           for k, v in com.items()}
    UENC = nc.dram_tensor("UENC", [BPC, 23, L], F32, kind="ExternalInput")
    UDEC = nc.dram_tensor("UDEC", [BPC, 23, L], F32, kind="ExternalInput")
    TRI = nc.dram_tensor("TRI", [BPC, CO, L], F32, kind="ExternalInput")
    OUT = nc.dram_tensor("OUT", [BPC, PRED, CO], F32, kind="ExternalOutput")
    ATT = [nc.dram_tensor(f"ATT{l}", [BPC, L, DM], F32, kind="ExternalOutput")
           for l in range(2)]
    dbg_outs = {}

    def dbgt(name, shape):
        if name in dbg:
            h = nc.dram_tensor("DBG_" + name, list(shape), F32,
                               kind="ExternalOutput")
            dbg_outs[name] = h
            return h
        return None

    with tile.TileContext(nc) as tc, ExitStack() as ctx:
        _build_body(nc, tc, ctx, inp, UENC, UDEC, TRI, OUT, ATT, upto, dbgt)
    nc.finalize()
    return nc, dbg_outs


def _build_body(nc, tc, ctx, inp, UENC, UDEC, TRI, OUT, ATT, upto, dbgt):
    cp = ctx.enter_context(tc.tile_pool(name="consts", bufs=1))
    dramp = ctx.enter_context(tc.tile_pool(name="spill", bufs=1, space="DRAM"))
    psp = [None]  # current psum pool, set per phase

    def cload(name, shape, src):
        t = cp.tile(list(shape), F32, name="c_" + name)
        nc.sync.dma_start(t[:], src)
        return t

    fcosA = cload("fcosA", (128, NF), inp["FCOS"][0:128, :])
    fcosB = cload("fcosB", (64, NF), inp["FCOS"][128:L, :])
    fsinA = cload("fsinA", (128, NF), inp["FSIN"][0:128, :])
    fsinB = cload("fsinB", (64, NF), inp["FSIN"][128:L, :])
    icos = cload("icos", (NF, L), inp["ICOS"][:, :])
    isin = cload("isin", (NF, L), inp["ISIN"][:, :])
    adtA = cload("adtA", (128, L), inp["ADT"][0:128, :])
    adtB = cload("adtB", (64, L), inp["ADT"][128:L, :])
    cenA = cload("cenA", (128, L), inp["CEN"][0:128, :])
    cenB = cload("cenB", (64, L), inp["CEN"][128:L, :])
    r0A = cload("r0A", (128, L), inp["R0"][0:128, :])
    r0B = cload("r0B", (64, L), inp["R0"][128:L, :])
    r2A = cload("r2A", (128, L), inp["R2"][0:128, :])
    r2B = cload("r2B", (64, L), inp["R2"][128:L, :])
    id1A = cload("id1A", (128, L), inp["ID192"][0:128, :])
    id1B = cload("id1B", (64, L), inp["ID192"][128:L, :])
    id16 = cload("id16", (16, 16), inp["ID16"][:, :])
    iota = cload("iota", (BPC, L), inp["IOTA"][:, :])
    embE = cload("embE", (23, DM), inp["EMB_E"][:, :])
    embD = cload("embD", (23, DM), inp["EMB_D"][:, :])
    epst = cp.tile([128, 1], F32, name="c_eps")
    nc.vector.memset(epst[:], 1e-5)

    def dtile(name, shape):
        return dramp.tile(list(shape), F32, name=name)

    xtc_e = [dtile(f"xtc_e{i}", (BPC, L, DM)) for i in range(3)]
    xct_e = [dtile(f"xct_e{i}", (BPC, DM, L)) for i in range(3)]
    xtc_d0 = dtile("xtc_d0", (BPC, L, DM))
    xct_d0 = dtile("xct_d0", (BPC, DM, L))
    x2tc_d = dtile("x2tc_d", (BPC, L, DM))
    x2ct_d = dtile("x2ct_d", (BPC, DM, L))
    tacc_d = dtile("tacc_d", (BPC, L, DM))
    vfre_b = dtile("vfre_b", (BPC, NF, DM))
    vfim_b = dtile("vfim_b", (BPC, NF, DM))
    kfcre_b = dtile("kfcre_b", (BPC, NF, DM))
    kfcim_b = dtile("kfcim_b", (BPC, NF, DM))
    vfcre_b = dtile("vfcre_b", (BPC, NF, DM))
    vfcim_b = dtile("vfcim_b", (BPC, NF, DM))

    ev_cnt = [0]

    def evict(dst, src):
        ev_cnt[0] += 1
        if ev_cnt[0] % 2 == 0:
            nc.scalar.copy(dst, src)
        else:
            nc.vector.tensor_copy(dst, src)

    def mm(ps_ap, pairs):
        n = len(pairs)
        for i, (lh, rh) in enumerate(pairs):
            nc.tensor.matmul(ps_ap, lhsT=lh, rhs=rh,
                             start=(i == 0), stop=(i == n - 1))

    _psn = [0]

    def pst(p, f, tag, bufs=None):
        _psn[0] += 1
        return psp[0].tile([p, f], F32, tag=tag, bufs=bufs,
                           name=f"{tag}_{_psn[0]}")

    def load_tc(pool, dram3, j, tag):
        a = pool.tile([128, DM], F32, tag=tag + "a", bufs=2)
        b = pool.tile([64, DM], F32, tag=tag + "b", bufs=2)
        nc.sync.dma_start(a[:], dram3[j, 0:128, :])
        nc.sync.dma_start(b[:], dram3[j, 128:L, :])
        return [a, b]

    def load_ct(pool, dram3, j, tag):
        out = []
        for ci, (cs, cl) in enumerate(CCH):
            t = pool.tile([128, L], F32, tag=f"{tag}{ci}", bufs=2)
            nc.sync.dma_start(t[:], dram3[j, cs:cs + cl, :])
            out.append(t)
        return out

    def store_tc(dram3, j, tcs):
        nc.sync.dma_start(dram3[j, 0:128, :], tcs[0][:])
        nc.sync.dma_start(dram3[j, 128:L, :], tcs[1][:])

    def store_ct(dram3, j, cts):
        for ci, (cs, cl) in enumerate(CCH):
            nc.sync.dma_start(dram3[j, cs:cs + cl, :], cts[ci][:])

    def proj_td(pool, xct, w4, tag):
        out = []
        for (ts, tl) in TCH:
            ps = pst(tl, DM, "tQ", 2)
            mm(ps[0:tl, :], [(xct[ci][:, ts:ts + tl], w4[ci][:])
                             for ci in range(4)])
            sb = pool.tile([tl, DM], F32, tag=f"{tag}{ts}")
            evict(sb[:], ps[0:tl, :])
            out.append(sb)
        return out

    def fft_of(ytd, tag):
        re = pst(NF, DM, "tF", 4)
        mm(re[0:NF, :], [(fcosA[:], ytd[0][:]), (fcosB[:], ytd[1][:])])
        im = pst(NF, DM, "tF", 4)
        mm(im[0:NF, :], [(fsinA[:], ytd[0][:]), (fsinB[:], ytd[1][:])])
        return re, im

    def ln_tc(pool, xtc, tag):
        out = []
        for i, (ts, tl) in enumerate(TCH):
            st = pool.tile([tl, 6], F32, tag=f"{tag}st{i}")
            nc.vector.bn_stats(st[:], xtc[i][:])
            mv2 = pool.tile([tl, 2], F32, tag=f"{tag}mv{i}")
            nc.vector.bn_aggr(mv2[:], st[:])
            rs = pool.tile([tl, 1], F32, tag=f"{tag}rs{i}")
            nc.scalar.activation(rs[:], mv2[:, 1:2], AF.Sqrt,
                                 bias=epst[0:tl, :])
            nc.vector.reciprocal(rs[:], rs[:])
            xh = pool.tile([tl, DM], F32, tag=f"{tag}xh{i}")
            nc.vector.tensor_scalar(xh[:], xtc[i][:], mv2[:, 0:1], rs[:],
                                    op0=OP.subtract, op1=OP.mult)
            out.append(xh)
        return out

    def rho_products(wp, qf, kf, rho_r, rho_i, j, need_p, tag):
        if need_p:
            m1 = wp.tile([NF, DM], F32, tag=tag + "m1")
            nc.gpsimd.tensor_tensor(m1[:], qf[0][:], kf[0][:], op=OP.mult)
            m2 = wp.tile([NF, DM], F32, tag=tag + "m2")
            nc.vector.tensor_tensor(m2[:], qf[1][:], kf[1][:], op=OP.mult)
            rep = wp.tile([NF, DM], F32, tag=tag + "re")
            nc.vector.scalar_tensor_tensor(
                rep[:], m1[:], 1.0, m2[:], op0=OP.mult, op1=OP.add,
                accum_out=rho_r[:, j:j + 1])
            m3 = wp.tile([NF, DM], F32, tag=tag + "m3")
            nc.gpsimd.tensor_tensor(m3[:], qf[1][:], kf[0][:], op=OP.mult)
            m4 = wp.tile([NF, DM], F32, tag=tag + "m4")
            nc.vector.tensor_tensor(m4[:], qf[0][:], kf[1][:], op=OP.mult)
            imp = wp.tile([NF, DM], F32, tag=tag + "im")
            nc.vector.scalar_tensor_tensor(
                imp[:], m3[:], 1.0, m4[:], op0=OP.mult, op1=OP.subtract,
                accum_out=rho_i[:, j:j + 1])
            return rep, imp
        m1 = wp.tile([NF, DM], F32, tag=tag + "m1")
        nc.gpsimd.tensor_tensor(m1[:], qf[0][:], kf[0][:], op=OP.mult)
        m2 = wp.tile([NF, DM], F32, tag=tag + "m2")
        nc.vector.tensor_tensor(m2[:], qf[1][:], kf[1][:], op=OP.mult)
        rep = wp.tile([NF, DM], F32, tag=tag + "re")
        nc.vector.scalar_tensor_tensor(
            rep[:], m1[:], 1.0, m2[:], op0=OP.mult, op1=OP.add,
            accum_out=rho_r[:, j:j + 1])
        m3 = wp.tile([NF, DM], F32, tag=tag + "m3")
        nc.gpsimd.tensor_tensor(m3[:], qf[1][:], kf[0][:], op=OP.mult)
        m4 = wp.tile([NF, DM], F32, tag=tag + "m4")
        nc.vector.tensor_tensor(m4[:], qf[0][:], kf[1][:], op=OP.mult)
        imp = wp.tile([NF, DM], F32, tag=tag + "im")
        nc.vector.scalar_tensor_tensor(
            imp[:], m3[:], 1.0, m4[:], op0=OP.mult, op1=OP.subtract,
            accum_out=rho_i[:, j:j + 1])
        return None, None

    def topk_block(wp, tkp, rho_r, rho_i, tag):
        mvp = pst(BPC, L, "tC", 2)
        mm(mvp[0:BPC, :], [(rho_r[:], icos[:]), (rho_i[:], isin[:])])
        mvs = wp.tile([BPC, L], F32, tag=tag + "mvs")
        nc.vector.tensor_copy(mvs[:], mvp[0:BPC, :])
        tv = wp.tile([BPC, 8], F32, tag=tag + "tv")
        nc.vector.max(tv[:], mvs[:])
        ti = wp.tile([BPC, 8], mybir.dt.uint32, tag=tag + "ti")
        nc.vector.max_index(ti[:], tv[:], mvs[:])
        tif = wp.tile([BPC, 8], F32, tag=tag + "tif")
        nc.vector.tensor_copy(tif[:], ti[:])
        negb = wp.tile([BPC, 1], F32, tag=tag + "negb")
        nc.vector.tensor_scalar_mul(negb[:], tv[:, 0:1], -1.0 / DM)
        ex = wp.tile([BPC, TOPK], F32, tag=tag + "ex")
        nc.scalar.activation(ex[:], tv[:, 0:TOPK], AF.Exp,
                             bias=negb[:], scale=1.0 / DM)
        ssum = wp.tile([BPC, 1], F32, tag=tag + "ss")
        nc.vector.tensor_reduce(ssum[:], ex[:], axis=AX.X, op=OP.add)
        nc.vector.reciprocal(ssum[:], ssum[:])
        w5 = wp.tile([BPC, TOPK], F32, tag=tag + "w5")
        nc.vector.tensor_scalar_mul(w5[:], ex[:], ssum[:])
        u = wp.tile([BPC, L], F32, tag=tag + "u")
        nc.vector.tensor_scalar(u[:], iota[:], tif[:, 0:1], w5[:, 0:1],
                                op0=OP.is_equal, op1=OP.mult)
        for k in range(1, TOPK):
            uk = wp.tile([BPC, L], F32, tag=tag + "uk")
            nc.vector.tensor_scalar(uk[:], iota[:], tif[:, k:k + 1],
                                    w5[:, k:k + 1],
                                    op0=OP.is_equal, op1=OP.mult)
            nc.vector.tensor_tensor(u[:], u[:], uk[:], op=OP.add)
        utp_a = pst(128, BPC, "tC", 2)
        mm(utp_a[0:128, :], [(u[:, 0:128], id16[:])])
        utp_b = pst(64, BPC, "tC", 2)
        mm(utp_b[0:64, :], [(u[:, 128:L], id16[:])])
        uta = wp.tile([128, BPC], F32, tag=tag + "usa")
        evict(uta[:], utp_a[0:128, :])
        utb = wp.tile([64, BPC], F32, tag=tag + "usb")
        evict(utb[:], utp_b[0:64, :])
        ufr_p = pst(NF, BPC, "tC", 2)
        mm(ufr_p[0:NF, :], [(fcosA[:], uta[:]), (fcosB[:], utb[:])])
        ufi_p = pst(NF, BPC, "tC", 2)
        mm(ufi_p[0:NF, :], [(fsinA[:], uta[:]), (fsinB[:], utb[:])])
        ufr = tkp.tile([NF, BPC], F32, tag=tag + "ufrs", bufs=1)
        evict(ufr[:], ufr_p[0:NF, :])
        ufi = tkp.tile([NF, BPC], F32, tag=tag + "ufis", bufs=1)
        evict(ufi[:], ufi_p[0:NF, :])
        return ufr, ufi, u

    def agg_ct(wp, vfre, vfim, ufr, ufi, j, tag):
        tmp1 = wp.tile([NF, L], F32, tag=tag + "t1")
        nc.vector.tensor_scalar_mul(tmp1[:], isin[:], ufi[:, j:j + 1])
        g1 = wp.tile([NF, L], F32, tag=tag + "g1")
        nc.vector.scalar_tensor_tensor(g1[:], icos[:], ufr[:, j:j + 1],
                                       tmp1[:], op0=OP.mult, op1=OP.subtract)
        tmp2 = wp.tile([NF, L], F32, tag=tag + "t2")
        nc.gpsimd.tensor_scalar_mul(tmp2[:], isin[:], ufr[:, j:j + 1])
        g2 = wp.tile([NF, L], F32, tag=tag + "g2")
        nc.vector.scalar_tensor_tensor(g2[:], icos[:], ufi[:, j:j + 1],
                                       tmp2[:], op0=OP.mult, op1=OP.add)
        outs = []
        for ci, (cs, cl) in enumerate(CCH):
            ps = pst(128, L, "tAG", 2)
            mm(ps[0:128, :], [(vfre[:, cs:cs + cl], g1[:]),
                              (vfim[:, cs:cs + cl], g2[:])])
            sb = wp.tile([128, L], F32, tag=f"{tag}as{ci}")
            evict(sb[:], ps[0:128, :])
            outs.append(sb)
        return outs

    def proj_from_ct(src_ct, w4, tag):
        outp = []
        for (ts, tl) in TCH:
            ps = pst(tl, DM, "tAO", 1)
            mm(ps[0:tl, :], [(src_ct[ci][:, ts:ts + tl], w4[ci][:])
                             for ci in range(4)])
            outp.append(ps)
        return outp

    def decomp_dual(wp, u_tc, tag, want_ct=True):
        tc_out = []
        for i, (ts, tl) in enumerate(TCH):
            ps = pst(tl, DM, "tD", 2)
            mm(ps[0:tl, :], [(adtA[:, ts:ts + tl], u_tc[0][:]),
                             (adtB[:, ts:ts + tl], u_tc[1][:])])
            sb = wp.tile([tl, DM], F32, tag=tag + f"tcs{i}")
            evict(sb[:], ps[0:tl, :])
            tc_out.append(sb)
        ct_out = None
        if want_ct:
            ct_out = []
            for ci, (cs, cl) in enumerate(CCH):
                ps = pst(128, L, "tD", 2)
                mm(ps[0:128, :], [(u_tc[0][:, cs:cs + cl], adtA[:]),
                                  (u_tc[1][:, cs:cs + cl], adtB[:])])
                sb = wp.tile([128, L], F32, tag=tag + f"cts{ci}")
                evict(sb[:], ps[0:128, :])
                ct_out.append(sb)
        return tc_out, ct_out

    def load_w4(pool, name, tag):
        w = []
        for ci, (cs, cl) in enumerate(CCH):
            t = pool.tile([128, DM], F32, tag=f"{tag}{ci}", bufs=1)
            nc.sync.dma_start(t[:], inp[name][cs:cs + cl, :])
            w.append(t)
        return w

    # ======================= STAGE 0: embeddings =======================
    d_x0e = dbgt("x0e", (BPC, L, DM))
    d_x0d = dbgt("x0d", (BPC, L, DM))
    with tc.tile_pool(name="s0", bufs=3) as wp, \
            tc.tile_pool(name="s0p", bufs=1, space="PSUM") as pp0:
        psp[0] = pp0
        for j in range(BPC):
            for (udram, xtcb, xctb, emb, dd) in (
                    (UENC, xtc_e[0], xct_e[0], embE, d_x0e),
                    (UDEC, xtc_d0, xct_d0, embD, d_x0d)):
                ue = wp.tile([23, L], F32, tag="ue")
                nc.sync.dma_start(ue[:], udram[j, :, :])
                tcs = []
                for i, (ts, tl) in enumerate(TCH):
                    ps = pst(tl, DM, "tQ", 2)
                    mm(ps[0:tl, :], [(ue[:, ts:ts + tl], emb[:])])
                    sb = wp.tile([tl, DM], F32, tag=f"embs{i}")
                    evict(sb[:], ps[0:tl, :])
                    tcs.append(sb)
                store_tc(xtcb, j, tcs)
                if dd is not None:
                    store_tc(dd, j, tcs)
                cts = []
                for ci, (cs, cl) in enumerate(CCH):
                    ps = pst(128, L, "tC", 2)
                    mm(ps[0:128, :], [(emb[:, cs:cs + cl], ue[:])])
                    sb = wp.tile([128, L], F32, tag=f"embc{ci}")
                    evict(sb[:], ps[0:128, :])
                    cts.append(sb)
                store_ct(xctb, j, cts)
    if upto < 1:
        return

    # ======================= ENCODER =======================
    def ac_loop_a(wp, xct_src, wq, wk, wv, rho_r, rho_i, j,
                  corr_dst, vf_spill, tag):
        xct = load_ct(wp, xct_src, j, tag + "x")
        q = proj_td(wp, xct, wq, tag + "q")
        k = proj_td(wp, xct, wk, tag + "k")
        v = proj_td(wp, xct, wv, tag + "v")
        qf_p = fft_of(q, tag + "qf")
        kf_p = fft_of(k, tag + "kf")
        vf_p = fft_of(v, tag + "vf")
        qf, kf = [], []
        for nm, p, lst in (("qr", qf_p[0], qf), ("qi", qf_p[1], qf),
                           ("kr", kf_p[0], kf), ("ki", kf_p[1], kf)):
            sb = wp.tile([NF, DM], F32, tag=tag + nm)
            evict(sb[:], p[0:NF, :])
            lst.append(sb)
        for nm, p, dst in (("vr", vf_p[0], vf_spill[0]),
                           ("vi", vf_p[1], vf_spill[1])):
            sb = wp.tile([NF, DM], F32, tag=tag + nm)
            evict(sb[:], p[0:NF, :])
            nc.sync.dma_start(dst[j, :, :], sb[:])
        rep, imp = rho_products(wp, qf, kf, rho_r, rho_i, j,
                                corr_dst is not None, tag + "p")
        if corr_dst is not None:
            for (ts, tl) in TCH:
                ps = pst(tl, DM, "tC", 2)
                mm(ps[0:tl, :], [(icos[:, ts:ts + tl], rep[:]),
                                 (isin[:, ts:ts + tl], imp[:])])
                sb = wp.tile([tl, DM], F32, tag=tag + f"cos{ts}")
                evict(sb[:], ps[0:tl, :])
                nc.sync.dma_start(corr_dst[j, ts:ts + tl, :], sb[:])

    def ffn_block(wp, w1, w2, x_ct, x_tc_res, tagp):
        h = []
        for di in range(16):
            ps = pst(128, L, "tH", 2)
            mm(ps[0:128, :],
               [(w1[ci][:, di * 128:(di + 1) * 128], x_ct[ci][:])
                for ci in range(4)])
            hs = wp.tile([128, L], F32, tag=f"{tagp}hs{di}")
            nc.scalar.activation(hs[:], ps[0:128, :], AF.Gelu)
            h.append(hs)
        u3 = []
        for i, (ts, tl) in enumerate(TCH):
            ps = pst(tl, DM, "tY", 1)
            mm(ps[0:tl, :], [(h[di][:, ts:ts + tl], w2[di][:])
                             for di in range(16)])
            sb = wp.tile([tl, DM], F32, tag=f"{tagp}u3{i}")
            nc.vector.tensor_tensor(sb[:], ps[0:tl, :], x_tc_res[i][:],
                                    op=OP.add)
            u3.append(sb)
        return u3

    def load_ffn_w(wp, w1name, w2name, tagp):
        w1 = []
        for ci, (cs, cl) in enumerate(CCH):
            t = wp.tile([128, DFF], F32, tag=f"{tagp}w1{ci}", bufs=1)
            nc.sync.dma_start(t[:], inp[w1name][cs:cs + cl, :])
            w1.append(t)
        w2 = []
        for di in range(16):
            t = wp.tile([128, DM], F32, tag=f"{tagp}w2{di}", bufs=1)
            nc.sync.dma_start(t[:], inp[w2name][di * 128:(di + 1) * 128, :])
            w2.append(t)
        return w1, w2

    def enc_layer(l, xin_tc, xin_ct, xout_tc, xout_ct, att_dst):
        d_x = dbgt(f"x{l + 1}e", (BPC, L, DM))
        d_u = dbgt(f"u{l}e", (BPC, L)) if l == 0 else None
        with ExitStack() as ls:
            tkp = ls.enter_context(tc.tile_pool(name=f"e{l}tk", bufs=1))
            with ExitStack() as lsa:
                wp = lsa.enter_context(tc.tile_pool(name=f"e{l}a", bufs=2))
                psp[0] = lsa.enter_context(
                    tc.tile_pool(name=f"e{l}ap", bufs=1, space="PSUM"))
                wq = load_w4(wp, f"e{l}_wq", "wwq")
                wk = load_w4(wp, f"e{l}_wk", "wwk")
                wv = load_w4(wp, f"e{l}_wv", "wwv")
                rho_r = tkp.tile([NF, BPC], F32, tag="rhor", bufs=1)
                rho_i = tkp.tile([NF, BPC], F32, tag="rhoi", bufs=1)
                for j in range(BPC):
                    ac_loop_a(wp, xin_ct, wq, wk, wv, rho_r, rho_i, j,
                              att_dst, (vfre_b, vfim_b), "A")
                ufr, ufi, u_dbg = topk_block(wp, tkp, rho_r, rho_i, f"e{l}tk")
                if d_u is not None:
                    nc.sync.dma_start(d_u[:, :], u_dbg[:])
            wp = ls.enter_context(tc.tile_pool(name=f"e{l}b", bufs=1))
            psp[0] = ls.enter_context(
                tc.tile_pool(name=f"e{l}bp", bufs=1, space="PSUM"))
            wo = load_w4(wp, f"e{l}_wo", "wwo")
            w1, w2 = load_ffn_w(wp, f"e{l}_w1", f"e{l}_w2", "B")
            for jp in range(BPC // 2):
                pair_ct = [wp.tile([128, 2 * L], F32, tag=f"Bpc{ci}",
                                   bufs=1, name=f"pairct{ci}")
                           for ci in range(4)]
                x2_tcs = []
                for hf in range(2):
                    j = 2 * jp + hf
                    x_tc = load_tc(wp, xin_tc, j, f"Bx{hf}")
                    vfre = wp.tile([NF, DM], F32, tag=f"Bvr{hf}", bufs=1)
                    nc.sync.dma_start(vfre[:], vfre_b[j, :, :])
                    vfim = wp.tile([NF, DM], F32, tag=f"Bvi{hf}", bufs=1)
                    nc.sync.dma_start(vfim[:], vfim_b[j, :, :])
                    agg = agg_ct(wp, vfre, vfim, ufr, ufi, j, f"Bg{hf}")
                    ao_p = proj_from_ct(agg, wo, f"Bao{hf}")
                    u1 = []
                    for i, (ts, tl) in enumerate(TCH):
                        sb = wp.tile([tl, DM], F32, tag=f"Bu1{hf}{i}")
                        nc.vector.tensor_tensor(sb[:], ao_p[i][0:tl, :],
                                                x_tc[i][:], op=OP.add)
                        u1.append(sb)
                    # decomp tc into per-half tiles; ct directly into pair_ct
                    x2_tc = []
                    for i, (ts, tl) in enumerate(TCH):
                        ps = pst(tl, DM, "tD", 2)
                        mm(ps[0:tl, :], [(adtA[:, ts:ts + tl], u1[0][:]),
                                         (adtB[:, ts:ts + tl], u1[1][:])])
                        sb = wp.tile([tl, DM], F32, tag=f"Bd1t{hf}{i}")
                        evict(sb[:], ps[0:tl, :])
                        x2_tc.append(sb)
                    for ci, (cs, cl) in enumerate(CCH):
                        ps = pst(128, L, "tD", 2)
                        mm(ps[0:128, :], [(u1[0][:, cs:cs + cl], adtA[:]),
                                          (u1[1][:, cs:cs + cl], adtB[:])])
                        evict(pair_ct[ci][:, hf * L:(hf + 1) * L],
                              ps[0:128, :])
                    x2_tcs.append(x2_tc)
                # paired FFN first half: h = gelu(x2 @ W1) for both samples
                h = []
                for di in range(16):
                    ps = pst(128, 2 * L, "tH", 2)
                    mm(ps[0:128, :],
                       [(w1[ci][:, di * 128:(di + 1) * 128], pair_ct[ci][:])
                        for ci in range(4)])
                    hs = wp.tile([128, 2 * L], F32, tag=f"Bhs{di}")
                    nc.scalar.activation(hs[:], ps[0:128, :], AF.Gelu)
                    h.append(hs)
                for hf in range(2):
                    j = 2 * jp + hf
                    off = hf * L
                    u3 = []
                    for i, (ts, tl) in enumerate(TCH):
                        ps = pst(tl, DM, "tY", 1)
                        mm(ps[0:tl, :],
                           [(h[di][:, off + ts:off + ts + tl], w2[di][:])
                            for di in range(16)])
                        sb = wp.tile([tl, DM], F32, tag=f"Bu3{hf}{i}")
                        nc.vector.tensor_tensor(sb[:], ps[0:tl, :],
                                                x2_tcs[hf][i][:], op=OP.add)
                        u3.append(sb)
                    x3_tc, x3_ct = decomp_dual(wp, u3, f"Bd2{hf}")
                    store_tc(xout_tc, j, x3_tc)
                    store_ct(xout_ct, j, x3_ct)
                    if d_x is not None:
                        store_tc(d_x, j, x3_tc)

    enc_layer(0, xtc_e[0], xct_e[0], xtc_e[1], xct_e[1], ATT[0])
    if upto < 2:
        return
    enc_layer(1, xtc_e[1], xct_e[1], xtc_e[2], xct_e[2], ATT[1])
    if upto < 3:
        return

    # ================ STAGE 3: enc_norm + cross K/V FFTs ================
    d_kfc = dbgt("kfc", (BPC, NF, DM))
    with tc.tile_pool(name="s3", bufs=2) as wp, \
            tc.tile_pool(name="s3p", bufs=1, space="PSUM") as pp3:
        psp[0] = pp3
        wkc = load_w4(wp, "dc_wk", "wkc")
        wvc = load_w4(wp, "dc_wv", "wvc")
        for j in range(BPC):
            x_tc = load_tc(wp, xtc_e[2], j, "Nx")
            xh = ln_tc(wp, x_tc, "Nln")
            cen_ct = []
            for ci, (cs, cl) in enumerate(CCH):
                ps = pst(128, L, "tC", 2)
                mm(ps[0:128, :], [(xh[0][:, cs:cs + cl], cenA[:]),
                                  (xh[1][:, cs:cs + cl], cenB[:])])
                sb = wp.tile([128, L], F32, tag=f"Ncs{ci}")
                evict(sb[:], ps[0:128, :])
                cen_ct.append(sb)
            for w4, spill in ((wkc, (kfcre_b, kfcim_b)),
                              (wvc, (vfcre_b, vfcim_b))):
                y = proj_td(wp, cen_ct, w4, "Nkv")
                re_p, im_p = fft_of(y, "Nf")
                for p, dst, nm in ((re_p, spill[0], "r"), (im_p, spill[1], "i")):
                    sb = wp.tile([NF, DM], F32, tag="Nfs" + nm)
                    evict(sb[:], p[0:NF, :])
                    nc.sync.dma_start(dst[j, :, :], sb[:])
                    if nm == "r" and spill[0] is kfcre_b and d_kfc is not None:
                        nc.sync.dma_start(d_kfc[j, :, :], sb[:])
    if upto < 4:
        return

    # ================ STAGE 4: decoder self-attention ================
    d_x2 = dbgt("x2d", (BPC, L, DM))
    with ExitStack() as ls4:
        tkp = ls4.enter_context(tc.tile_pool(name="s4tk", bufs=1))
        with ExitStack() as ls4a:
            wp = ls4a.enter_context(tc.tile_pool(name="s4a", bufs=2))
            psp[0] = ls4a.enter_context(
                tc.tile_pool(name="s4ap", bufs=1, space="PSUM"))
            wq = load_w4(wp, "ds_wq", "dwq")
            wk = load_w4(wp, "ds_wk", "dwk")
            wv = load_w4(wp, "ds_wv", "dwv")
            rho_r = tkp.tile([NF, BPC], F32, tag="rhor2", bufs=1)
            rho_i = tkp.tile([NF, BPC], F32, tag="rhoi2", bufs=1)
            for j in range(BPC):
                ac_loop_a(wp, xct_d0, wq, wk, wv, rho_r, rho_i, j,
                          None, (vfre_b, vfim_b), "D")
            ufr, ufi, _ = topk_block(wp, tkp, rho_r, rho_i, "dtk")
        wp = ls4.enter_context(tc.tile_pool(name="s4b", bufs=1))
        psp[0] = ls4.enter_context(
            tc.tile_pool(name="s4bp", bufs=1, space="PSUM"))
        wo = load_w4(wp, "ds_wo", "dwo")
        for j in range(BPC):
            x_tc = load_tc(wp, xtc_d0, j, "Dx")
            vfre = wp.tile([NF, DM], F32, tag="Dvr", bufs=2)
            nc.sync.dma_start(vfre[:], vfre_b[j, :, :])
            vfim = wp.tile([NF, DM], F32, tag="Dvi", bufs=2)
            nc.sync.dma_start(vfim[:], vfim_b[j, :, :])
            agg = agg_ct(wp, vfre, vfim, ufr, ufi, j, "Dg")
            ao_p = proj_from_ct(agg, wo, "Dao")
            u1 = []
            for i, (ts, tl) in enumerate(TCH):
                sb = wp.tile([tl, DM], F32, tag=f"Du1{i}")
                nc.vector.tensor_tensor(sb[:], ao_p[i][0:tl, :],
                                        x_tc[i][:], op=OP.add)
                u1.append(sb)
            x2_tc, x2_ct = decomp_dual(wp, u1, "Dd1")
            tacc = []
            for i in range(2):
                sb = wp.tile([TCH[i][1], DM], F32, tag=f"Dt{i}")
                nc.vector.tensor_tensor(sb[:], u1[i][:], x2_tc[i][:],
                                        op=OP.subtract)
                tacc.append(sb)
            store_tc(x2tc_d, j, x2_tc)
            store_ct(x2ct_d, j, x2_ct)
            store_tc(tacc_d, j, tacc)
            if d_x2 is not None:
                store_tc(d_x2, j, x2_tc)
    if upto < 5:
        return

    # ================ STAGE 5/6/7: cross attn + FFN + head ================
    with ExitStack() as ls5:
        tkp = ls5.enter_context(tc.tile_pool(name="s5tk", bufs=1))
        ls5a = ExitStack()
        wp = ls5a.enter_context(tc.tile_pool(name="s5a", bufs=2))
        psp[0] = ls5a.enter_context(
            tc.tile_pool(name="s5ap", bufs=1, space="PSUM"))
        cwq = load_w4(wp, "dc_wq", "cwq")
        rho_r = tkp.tile([NF, BPC], F32, tag="rhor3", bufs=1)
        rho_i = tkp.tile([NF, BPC], F32, tag="rhoi3", bufs=1)
        for j in range(BPC):
            xct = load_ct(wp, x2ct_d, j, "Cx")
            q = proj_td(wp, xct, cwq, "Cq")
            qf_p = fft_of(q, "Cqf")
            qf = []
            for nm, p in (("qr", qf_p[0]), ("qi", qf_p[1])):
                sb = wp.tile([NF, DM], F32, tag="C" + nm)
                evict(sb[:], p[0:NF, :])
                qf.append(sb)
            kfr = wp.tile([NF, DM], F32, tag="Ckr", bufs=2)
            nc.sync.dma_start(kfr[:], kfcre_b[j, :, :])
            kfi = wp.tile([NF, DM], F32, tag="Cki", bufs=2)
            nc.sync.dma_start(kfi[:], kfcim_b[j, :, :])
            rho_products(wp, qf, (kfr, kfi), rho_r, rho_i, j, False, "Cp")
        ufr3, ufi3, _ = topk_block(wp, tkp, rho_r, rho_i, "ctk")
        ls5a.close()
        wp = ls5.enter_context(tc.tile_pool(name="s5b", bufs=1))
        psp[0] = ls5.enter_context(
            tc.tile_pool(name="s5bp", bufs=1, space="PSUM"))
        cwo = load_w4(wp, "dc_wo", "cwo")
        w1, w2 = load_ffn_w(wp, "d_w1", "d_w2", "F")
        wtj = []
        for jj in range(3):
            per = []
            for ci, (cs, cl) in enumerate(CCH):
                t = wp.tile([128, CO], F32, tag=f"dwt{jj}{ci}", bufs=1)
                nc.sync.dma_start(t[:], inp["d_wt"][jj, cs:cs + cl, :])
                per.append(t)
            wtj.append(per)
        wpp = []
        for ci, (cs, cl) in enumerate(CCH):
            t = wp.tile([128, CO], F32, tag=f"wpp{ci}", bufs=1)
            nc.sync.dma_start(t[:], inp["WPP"][cs:cs + cl, :])
            wpp.append(t)

        if upto < 6:
            return

        shift_consts = ((r0A, r0B), (id1A, id1B), (r2A, r2B))
        for j in range(BPC):
            x2_tc = load_tc(wp, x2tc_d, j, "Fx2")
            tacc = load_tc(wp, tacc_d, j, "Fta")
            vfre = wp.tile([NF, DM], F32, tag="Fvr", bufs=2)
            nc.sync.dma_start(vfre[:], vfcre_b[j, :, :])
            vfim = wp.tile([NF, DM], F32, tag="Fvi", bufs=2)
            nc.sync.dma_start(vfim[:], vfcim_b[j, :, :])
            agg = agg_ct(wp, vfre, vfim, ufr3, ufi3, j, "Fg")
            ca_p = proj_from_ct(agg, cwo, "Fca")
            u2 = []
            for i, (ts, tl) in enumerate(TCH):
                sb = wp.tile([tl, DM], F32, tag=f"Fu2{i}")
                nc.vector.tensor_tensor(sb[:], ca_p[i][0:tl, :],
                                        x2_tc[i][:], op=OP.add)
                u2.append(sb)
            x3_tc, x3_ct = decomp_dual(wp, u2, "Fd2")
            for i in range(2):
                tmp = wp.tile([TCH[i][1], DM], F32, tag=f"Ftm{i}")
                nc.vector.tensor_tensor(tmp[:], u2[i][:], x3_tc[i][:],
                                        op=OP.subtract)
                nc.vector.tensor_tensor(tacc[i][:], tacc[i][:], tmp[:],
                                        op=OP.add)
            u3 = ffn_block(wp, w1, w2, x3_ct, x3_tc, "F")
            x4_tc, _ = decomp_dual(wp, u3, "Fd3", want_ct=False)
            for i in range(2):
                tmp = wp.tile([TCH[i][1], DM], F32, tag=f"Ftn{i}")
                nc.vector.tensor_tensor(tmp[:], u3[i][:], x4_tc[i][:],
                                        op=OP.subtract)
                nc.vector.tensor_tensor(tacc[i][:], tacc[i][:], tmp[:],
                                        op=OP.add)
            xh = ln_tc(wp, x4_tc, "Fln")
            cen_ct = []
            for ci, (cs, cl) in enumerate(CCH):
                ps = pst(128, L, "tD", 2)
                mm(ps[0:128, :], [(xh[0][:, cs:cs + cl], cenA[:]),
                                  (xh[1][:, cs:cs + cl], cenB[:])])
                sb = wp.tile([128, L], F32, tag=f"Fcs{ci}")
                evict(sb[:], ps[0:128, :])
                cen_ct.append(sb)
            seas_p = pst(CO, L, "tY", 1)
            mm(seas_p[0:CO, :], [(wpp[ci][:], cen_ct[ci][:])
                                 for ci in range(4)])
            sj_all = []
            for jj in range(3):
                rA, rB = shift_consts[jj]
                sjc = []
                for ci, (cs, cl) in enumerate(CCH):
                    ps = pst(128, L, "tAG", 2)
                    mm(ps[0:128, :], [(tacc[0][:, cs:cs + cl], rA[:]),
                                      (tacc[1][:, cs:cs + cl], rB[:])])
                    sb = wp.tile([128, L], F32, tag=f"Fss{jj}{ci}")
                    evict(sb[:], ps[0:128, :])
                    sjc.append(sb)
                sj_all.append(sjc)
            res_p = pst(CO, L, "tH", 2)
            mm(res_p[0:CO, :], [(wtj[jj][ci][:], sj_all[jj][ci][:])
                                for jj in range(3) for ci in range(4)])
            tri = wp.tile([CO, L], F32, tag="Ftri")
            nc.sync.dma_start(tri[:], TRI[j, :, :])
            o1 = wp.tile([CO, L], F32, tag="Fo1")
            nc.vector.tensor_tensor(o1[:], seas_p[0:CO, :], tri[:], op=OP.add)
            o2 = wp.tile([CO, L], F32, tag="Fo2")
            nc.vector.tensor_tensor(o2[:], res_p[0:CO, :], o1[:], op=OP.add)
            nc.sync.dma_start(OUT[j, :, :].rearrange("t c -> c t"),
                              o2[:, LBL:L])


_CACHE = {}


def _build_runner(com):
    """Compile once; return a callable in_maps -> per-core outputs, plus a
    raw executor handle for timing."""
    import jax
    from jax.sharding import Mesh, PartitionSpec, NamedSharding
    from jax.experimental.shard_map import shard_map
    import concourse.bass2jax as b2j

    nc, _ = build_nc(com)
    b2j.install_neuronx_cc_hook()
    partition_name = (nc.partition_id_tensor.name
                      if nc.partition_id_tensor else None)
    in_names, out_names, out_avals, zero_outs = [], [], [], []
    for alloc in nc.m.functions[0].allocations:
        if not isinstance(alloc, mybir.MemoryLocationSet):
            continue
        name = alloc.memorylocations[0].name
        if alloc.kind == "ExternalInput":
            if name != partition_name:
                in_names.append(name)
        elif alloc.kind == "ExternalOutput":
            shape = tuple(alloc.tensor_shape)
            dtype = mybir.dt.np(alloc.dtype)
            out_names.append(name)
            out_avals.append(jax.core.ShapedArray(shape, dtype))
            zero_outs.append(np.zeros(shape, dtype))
    all_in = list(in_names) + list(out_names)
    if partition_name is not None:
        all_in.append(partition_name)

    def _body(*args):
        operands = list(args)
        if partition_name is not None:
            operands.append(b2j.partition_id_tensor())
        return tuple(b2j._bass_exec_p.bind(
            *operands, out_avals=tuple(out_avals),
            in_names=tuple(all_in), out_names=tuple(out_names),
            lowering_input_output_aliases=(), sim_require_finite=True,
            sim_require_nnan=True, nc=nc))

    devices = jax.devices()[:NCORES]
    mesh = Mesh(np.asarray(devices), ("core",))
    nin = len(in_names) + len(out_names)
    fn = jax.jit(shard_map(_body, mesh=mesh,
                           in_specs=(PartitionSpec("core"),) * nin,
                           out_specs=(PartitionSpec("core"),) * len(out_names),
                           check_rep=False), keep_unused=True)
    sh = NamedSharding(mesh, PartitionSpec("core"))
    return dict(fn=fn, in_names=in_names, out_names=out_names,
                zero_outs=zero_outs, sh=sh)


def _run(com, per_core):
    import jax
    if "runner" not in _CACHE:
        _CACHE["runner"] = _build_runner(com)
    R = _CACHE["runner"]
    in_maps = []
    for c in range(NCORES):
        m = dict(com)
        m.update(per_core[c])
        in_maps.append(m)
    concat_in = [np.concatenate([np.asarray(in_maps[c][nm])
                                 for c in range(NCORES)], axis=0)
                 for nm in R["in_names"]]
    concat_zero = [np.zeros((NCORES * z.shape[0], *z.shape[1:]), z.dtype)
                   for z in R["zero_outs"]]
    dev_in = [jax.device_put(a, R["sh"]) for a in concat_in + concat_zero]
    _CACHE["dev_in"] = dev_in
    outs = R["fn"](*dev_in)
    jax.block_until_ready(outs)
    res = {nm: np.asarray(o) for nm, o in zip(R["out_names"], outs)}
    return res


def kernel(x_enc, x_mark_enc, x_dec, x_mark_dec, params):
    hc = _host_consts()
    com, per_core = _prep_host(x_enc, x_mark_enc, x_dec, x_mark_dec, params, hc)
    res = _run(com, per_core)
    out = res["OUT"].reshape(NCORES * BPC, PRED, CO)
    att0 = res["ATT0"].reshape(NCORES * BPC, L, DM)
    att1 = res["ATT1"].reshape(NCORES * BPC, L, DM)
    B = out.shape[0]
    return (out.astype(np.float32),
            att0.reshape(B, L, 8, 64).astype(np.float32),
            att1.reshape(B, L, 8, 64).astype(np.float32))


def bench_exec_ns(reps=20):
    """Amortized per-execution time of the cached runner (call kernel() first)."""
    import jax, time as _t
    R = _CACHE["runner"]
    dev_in = _CACHE["dev_in"]
    for _ in range(3):
        outs = R["fn"](*dev_in)
    jax.block_until_ready(outs)
    t0 = _t.time()
    allouts = [R["fn"](*dev_in) for _ in range(reps)]
    jax.block_until_ready(allouts)
    return (_t.time() - t0) / reps * 1e9
